# revision 1
# baseline (speedup 1.0000x reference)
"""Channel-attention ("attention transpose") Trainium2 kernel.

Reference computation (per batch b):
    attn = softmax(scale * x1[b].T @ x2[b], axis=-1)   # [C, C]
    out[b] = attn @ x2[b].T                            # [C, N]

Sharding: data-parallel over batch B=8 across the 8 NeuronCores; each core
computes one batch independently (no collectives).

Per-core plan:
  Loads: x2 staged fp32 chunk-per-n-tile, rounded to fp32r (GpSimd) into a
  resident [128, 32, 768] SBUF image; x1 c-blocks staged and rounded to
  fp32r (DVE). fp32r matmuls run at bf16 rate (free dim >= 256) with ~2^-14
  relative rounding error (measured on HW) vs bf16's 2^-9.
  Phase 1 (logits + softmax): c-tiles processed in PAIRS, n-chunk-major, so
  the first pair's accumulation interleaves with x2 chunk arrivals instead
  of serializing the whole phase behind the load.
    - attn[c-tile, :] accumulated in PSUM fp32 (fp32r matmuls).
    - softmax fused: DVE reduce_max over PSUM, ACT exp(scale*z - scale*max)
      PSUM->SBUF (bf16) with accumulated row sum, DVE reciprocal.
    - attnT[d, c-slice] produced right after each c softmax by transpose
      matmuls (lhsT @ identity) + PSUM->SBUF cast copies.
  Phase 2, n-panel outer (8 panels of 512): bf16 compute.
    - x2 panel cast to bf16 (GpSimd), transposed into x2T [d-part, n] by
      transpose matmuls, copies split DVE/ACT,
    - out[c-tile, panel] = sum_d attnT[d, c-tile].T @ x2T[d, panel],
    - ACT copy-out multiplies by the softmax reciprocal (per-partition).
"""

import numpy as np

import concourse.bass as bass
import concourse.mybir as mybir
import concourse.tile as tile
from concourse import bacc
from concourse.bass_utils import run_bass_kernel_spmd
from concourse.masks import make_identity

B, N, C = 8, 4096, 768
SCALE = (C // 8) ** -0.5  # 96^-0.5
P = 128
NO = N // P       # 32 n-tiles
CT = C // P       # 6 c-tiles
NPAN = 8          # phase-2 n panels
PAN = N // NPAN   # 512
TPP = PAN // P    # 4 transposes per panel per d-tile
DH = C // 2       # 384, d-half free width for phase-1 matmuls
X1CH = 8          # x1 c-block load chunks

# Tunables (overridable before build for A/B experiments)
CONFIG = {
    "p1_group": 2,    # c-tiles accumulated concurrently in phase 1
    "tr_batch": True, # batch 4 transposes per PSUM bank before copy-out
    "x1_bf16": False, # phase-1 lhsT in bf16 (FWL weight loads) vs fp32r
}

F32 = mybir.dt.float32
F32R = mybir.dt.float32r
BF16 = mybir.dt.bfloat16
AX = mybir.AxisListType
AF = mybir.ActivationFunctionType


def load_x1_block(nc, x1blk_pool, x1st_pool, x1, c):
    dt = BF16 if CONFIG["x1_bf16"] else F32R
    x1_blk = x1blk_pool.tile([P, NO, P], dt, tag="x1_blk", name=f"x1b{c}")
    x1_t = x1[:, c * P:(c + 1) * P].rearrange("(no p) c -> p no c", p=P)
    nqc = NO // X1CH
    for q in range(X1CH):
        st = x1st_pool.tile([P, nqc, P], F32, tag="x1_st",
                            name=f"x1s{c}_{q}")
        nc.sync.dma_start(out=st, in_=x1_t[:, q * nqc:(q + 1) * nqc, :])
        nc.vector.tensor_copy(out=x1_blk[:, q * nqc:(q + 1) * nqc, :],
                              in_=st)
    return x1_blk


def build_body(nc, tc, pools, identity_bf, x1, x2, out, variant="full"):
    (singles, stage_pool, attnc_pool, x1blk_pool, x1st_pool, x2bf_pool,
     x2t_pool, opool, stats, ps_attn, ps_misc) = pools

    x2_sb = singles.tile([P, NO, C], F32R, tag="x2_sb")
    attnT = singles.tile([P, CT, C], BF16, tag="attnT")
    recip = singles.tile([P, CT], F32, tag="recip")

    G = CONFIG["p1_group"]
    # x1 blocks for the first c-group are loaded ahead of x2 so the
    # group-0 accumulation can chase the x2 chunk arrivals.
    x1_blocks = {}
    for c in range(G):
        x1_blocks[c] = load_x1_block(nc, x1blk_pool, x1st_pool, x1, c)

    # x2 resident load: one fp32 staged chunk per n-tile, rounded to fp32r
    # on POOL. Phase-1 matmuls chase the chunk arrivals.
    x2_t = x2.rearrange("(no p) d -> p no d", p=P)
    for ch in range(NO):
        st = stage_pool.tile([P, C], F32, tag="x2_stage", name=f"x2s{ch}")
        nc.sync.dma_start(out=st, in_=x2_t[:, ch, :])
        nc.gpsimd.tensor_copy(out=x2_sb[:, ch, :], in_=st)

    # ---- Phase 1: logits + softmax, c-groups x chunk-major ----
    for g in range(CT // G):
        cs = tuple(range(G * g, G * g + G))
        ps_tiles = {}
        for c in cs:
            ps_tiles[c] = ps_attn.tile([P, 2, 512], F32, tag="ps_attn",
                                       name=f"ps{c}")
        if g == 0:
            order = [(n, c) for n in range(NO) for c in cs]
        else:
            order = [(n, c) for c in cs for n in range(NO)]
        for n, c in order:
            lhsT = x1_blocks[c][:, n, :]
            ps = ps_tiles[c]
            nc.tensor.matmul(ps[:, 0, :DH], lhsT, x2_sb[:, n, 0:DH],
                             start=(n == 0), stop=(n == NO - 1))
            nc.tensor.matmul(ps[:, 1, :DH], lhsT,
                             x2_sb[:, n, DH:2 * DH],
                             start=(n == 0), stop=(n == NO - 1))
        # prefetch next group's x1 blocks
        for c2 in range(G * g + G, G * g + 2 * G):
            if c2 < CT:
                x1_blocks[c2] = load_x1_block(nc, x1blk_pool, x1st_pool,
                                              x1, c2)
        # softmax + attnT slices for this group
        for c in cs:
            ps = ps_tiles[c]
            m = stats.tile([P, 1], F32, tag="m", name=f"m{c}")
            nc.vector.reduce_max(out=m, in_=ps[:, :, :DH], axis=AX.XY)
            nm = stats.tile([P, 1], F32, tag="nm", name=f"nm{c}")
            nc.vector.tensor_scalar_mul(nm, m, -SCALE)
            ssum = stats.tile([P, 2], F32, tag="ssum", name=f"ss{c}")
            attn_c = attnc_pool.tile([P, C], BF16, tag="attn_c",
                                     name=f"ac{c}")
            for h in range(2):
                nc.scalar.activation(
                    out=attn_c[:, h * DH:(h + 1) * DH],
                    in_=ps[:, h, :DH],
                    func=AF.Exp,
                    bias=nm,
                    scale=SCALE,
                    accum_out=ssum[:, h:h + 1],
                )
            stot = stats.tile([P, 1], F32, tag="stot", name=f"st{c}")
            nc.vector.reduce_sum(out=stot, in_=ssum, axis=AX.X)
            nc.vector.reciprocal(out=recip[:, c:c + 1], in_=stot)
            for db in range(0, CT, 4):
                nd = min(4, CT - db)
                pt = ps_misc.tile([P, 512], F32, tag="ps_misc",
                                  name=f"ptA{c}_{db}")
                for j in range(nd):
                    d = db + j
                    nc.tensor.matmul(pt[:, j * P:(j + 1) * P],
                                     attn_c[:, d * P:(d + 1) * P],
                                     identity_bf, start=True, stop=True)
                # one batched copy-out, strided across the CT dim of attnT
                dst = attnT[:, db:db + nd, c * P:(c + 1) * P]
                if (c + db) % 2 == 0:
                    nc.vector.tensor_copy(
                        out=dst, in_=pt[:, :nd * P].rearrange(
                            "p (j q) -> p j q", j=nd))
                else:
                    nc.scalar.copy(
                        out=dst, in_=pt[:, :nd * P].rearrange(
                            "p (j q) -> p j q", j=nd))

    # ---- Phase 2: out[c, n] = sum_d attnT[d, c] * x2T[d, n], scaled ----
    for pan in range(NPAN):
        x2bf = x2bf_pool.tile([P, TPP, C], BF16, tag="x2bf",
                              name=f"xb{pan}")
        nc.gpsimd.tensor_copy(
            out=x2bf,
            in_=x2_sb[:, pan * TPP:(pan + 1) * TPP, :].bitcast(F32))
        x2T = x2t_pool.tile([P, CT, PAN], BF16, tag="x2T", name=f"xt{pan}")
        for d in range(CT):
            pt = ps_misc.tile([P, 512], F32, tag="ps_misc",
                              name=f"ptB{pan}_{d}")
            for t in range(TPP):
                nc.tensor.matmul(pt[:, t * P:(t + 1) * P],
                                 x2bf[:, t, d * P:(d + 1) * P],
                                 identity_bf, start=True, stop=True)
            if d % 2 == 0:
                nc.vector.tensor_copy(out=x2T[:, d, :], in_=pt)
            else:
                nc.scalar.copy(out=x2T[:, d, :], in_=pt)
        for c in range(CT):
            ob = opool.tile([P, PAN], F32, tag="ob", name=f"ob{pan}_{c}")
            po = ps_misc.tile([P, PAN], F32, tag="ps_misc",
                              name=f"po{pan}_{c}")
            for d in range(CT):
                nc.tensor.matmul(po, attnT[:, d, c * P:(c + 1) * P],
                                 x2T[:, d, :],
                                 start=(d == 0), stop=(d == CT - 1))
            nc.scalar.activation(out=ob, in_=po, func=AF.Copy,
                                 scale=recip[:, c:c + 1])
            nc.sync.dma_start(
                out=out[c * P:(c + 1) * P, pan * PAN:(pan + 1) * PAN],
                in_=ob)


def build_kernel(reps=1, variant="full"):
    nc = bacc.Bacc("TRN2", target_bir_lowering=False, debug=False,
                   num_devices=8)
    x1 = nc.declare_dram_parameter("x_1", [N, C], F32, isOutput=False)
    x2 = nc.declare_dram_parameter("x_2", [N, C], F32, isOutput=False)
    out = nc.declare_dram_parameter("out", [C, N], F32, isOutput=True)

    with tile.TileContext(nc) as tc:
        with (
            tc.tile_pool(name="singles", bufs=1) as singles,
            tc.tile_pool(name="stage", bufs=2) as stage_pool,
            tc.tile_pool(name="attnc", bufs=2) as attnc_pool,
            tc.tile_pool(name="x1blk", bufs=4) as x1blk_pool,
            tc.tile_pool(name="x1st", bufs=2) as x1st_pool,
            tc.tile_pool(name="x2bf", bufs=1) as x2bf_pool,
            tc.tile_pool(name="x2t", bufs=2) as x2t_pool,
            tc.tile_pool(name="opool", bufs=2) as opool,
            tc.tile_pool(name="stats", bufs=4) as stats,
            tc.tile_pool(name="ps_attn", bufs=CONFIG["p1_group"],
                         space="PSUM") as ps_attn,
            tc.tile_pool(name="ps_misc", bufs=8 - 2 * CONFIG["p1_group"],
                         space="PSUM") as ps_misc,
        ):
            pools = (singles, stage_pool, attnc_pool, x1blk_pool, x1st_pool,
                     x2bf_pool, x2t_pool, opool, stats, ps_attn, ps_misc)
            identity_bf = singles.tile([P, P], BF16, tag="identity_bf")
            make_identity(nc, identity_bf)
            for _ in range(reps):
                build_body(nc, tc, pools, identity_bf, x1[:], x2[:], out[:],
                           variant=variant)
    nc.compile()
    return nc


_nc_cache = {}


def get_kernel(reps=1, variant="full"):
    key = (reps, variant)
    if key not in _nc_cache:
        _nc_cache[key] = build_kernel(reps, variant)
    return _nc_cache[key]


def kernel(x_1, x_2):
    x_1 = np.asarray(x_1, dtype=np.float32)
    x_2 = np.asarray(x_2, dtype=np.float32)
    assert x_1.shape == (B, N, C) and x_2.shape == (B, N, C)
    nc = get_kernel(reps=1)
    core_ids = list(range(8))
    in_maps = [
        {"x_1": np.ascontiguousarray(x_1[b]),
         "x_2": np.ascontiguousarray(x_2[b])}
        for b in core_ids
    ]
    res = run_bass_kernel_spmd(nc, in_maps, core_ids)
    return np.stack([res.results[b]["out"] for b in core_ids], axis=0)

